# revision 28
# baseline (speedup 1.0000x reference)
"""Trainium2 Bass kernel for nn_EuclideanDeconf (retrieval_knn).

Computes out = -mean((x[:, :, None] - W.T[None, :, :])**2, axis=1)
            = (2*x@W.T - ||x||^2 - ||w||^2) / D

Sharding: data-parallel over batch across 8 NeuronCores (512 rows each),
W replicated. Per core the kernel computes out^T [C, B_sh] (the host
transposes back):
  - x loaded fp32 (for exact ||x||^2), cast to bf16, PE-transposed into a
    resident d-major xT [128, 32, 512] used as the matmul moving operand
  - W streamed: cast to bf16, each 128x128 block PE-transposed and used
    immediately as the stationary operand (no resident W^T)
  - cross term accumulated in fp32 PSUM as out^T tiles [128 c, 512 b]
  - epilogue per c-tile: (2/D)*psum - ||w_c||^2/D (per-partition scalar)
    - ||x||^2/D (broadcast row, built once via a tiny DRAM roundtrip +
    gpsimd partition_broadcast)
"""

import os

import ml_dtypes
import numpy as np

B, D, C = 4096, 4096, 1024
NCORES = 8
P = 128
KG = 4         # transpose k-chunks per PSUM batch (x side)
LOOKAHEAD = 6  # W-chunk pipeline depth before its matmuls

_nc_cache = {}


def _build_bass(b_sh, d, c):
    import concourse.bacc as bacc
    import concourse.mybir as mybir
    import concourse.tile as tile

    f32 = mybir.dt.float32
    bf16 = mybir.dt.bfloat16
    AF = mybir.ActivationFunctionType

    BT = b_sh // P    # b-tiles per core
    KC = d // P       # contraction chunks
    CT = c // P       # c-tiles
    NS = CT // 2      # c-tile pairs (one pass each)
    NKG = KC // KG
    HD = d // 2

    nc = bacc.Bacc(trn_type="TRN2")
    x_d = nc.dram_tensor("x", [b_sh, d], f32, kind="ExternalInput")
    w_d = nc.dram_tensor("W", [c, d], f32, kind="ExternalInput")
    id_d = nc.dram_tensor("ident", [P, P], bf16, kind="ExternalInput")
    o_d = nc.dram_tensor("out", [c, b_sh], f32, kind="ExternalOutput")

    with tile.TileContext(nc) as tc:
        with (
            tc.tile_pool(name="const", bufs=1) as constp,
            tc.tile_pool(name="persist", bufs=1) as persist,
            tc.tile_pool(name="xstage", bufs=2) as xstage,
            tc.tile_pool(name="wstage", bufs=4) as wstage,
            tc.tile_pool(name="bfs", bufs=5) as bfs,
            tc.tile_pool(name="wtp", bufs=4) as wtp,
            tc.tile_pool(name="outp", bufs=3) as outp,
            tc.tile_pool(name="trp", bufs=4, space="PSUM") as trp,
            tc.tile_pool(name="mmp", bufs=4, space="PSUM") as mmp,
            tc.tile_pool(name="dramp", bufs=2, space="DRAM") as dramp,
        ):
            ident = constp.tile([P, P], bf16)
            with tc.high_priority():
                nc.sync.dma_start(ident[:, :], id_d[:, :])
            ws_negD = constp.tile([P, CT], f32)   # -||w_c||^2/D, c-partition
            xs_cols = constp.tile([P, BT], f32)   # ||x_b||^2, b-partition
            xsrow = constp.tile([1, b_sh], f32)   # -||x||^2/D, b-major row
            xs_rep = persist.tile([P, b_sh], f32)  # xsrow on all partitions
            xT = persist.tile([P, KC, b_sh], bf16)

            # ---- x: load, square, cast, transpose into resident xT ----
            for bt in range(BT):
                with tc.high_priority():
                    xs = xstage.tile([P, d], f32, tag="xs")
                    nc.sync.dma_start(xs, x_d[bt * P : (bt + 1) * P, :])
                    xb = bfs.tile([P, d], bf16, tag="bfs")
                    nc.vector.tensor_copy(out=xb, in_=xs)
                    # square in place (fp32 dead after), free-dim sum
                    nc.scalar.activation(
                        xs, xs, AF.Square, accum_out=xs_cols[:, bt : bt + 1]
                    )
                for kg in range(NKG):
                    pt = trp.tile([P, KG, P], bf16, tag="tr")
                    for j in range(KG):
                        kk = kg * KG + j
                        nc.tensor.transpose(
                            pt[:, j, :], xb[:, kk * P : (kk + 1) * P], ident
                        )
                    dst = xT[:, kg * KG : (kg + 1) * KG, bt * P : (bt + 1) * P]
                    nc.vector.tensor_copy(out=dst, in_=pt[:, :, :])

            # ---- xs row: cols -> -xs/D -> DRAM -> row -> broadcast ----
            with tc.high_priority():
                nc.vector.tensor_scalar_mul(xs_cols, xs_cols, -1.0 / d)
                dtmp = dramp.tile([BT, P], f32)
                for t in range(BT):
                    nc.gpsimd.dma_start(dtmp[t, :], xs_cols[:, t : t + 1])
                nc.gpsimd.dma_start(xsrow[0:1, :], dtmp[:, :])
                nc.gpsimd.partition_broadcast(xs_rep[:, :], xsrow[0:1, :])

            # ---- W: stream pair-of-c-tiles passes ----
            def load_w_tile(ct):
                wb = bfs.tile([P, d], bf16, tag="bfs", name=f"wb{ct}")
                for h in range(2):
                    with tc.high_priority():
                        ws_ = wstage.tile([P, HD], f32, tag="ws")
                        nc.sync.dma_start(
                            ws_, w_d[ct * P : (ct + 1) * P, h * HD : (h + 1) * HD]
                        )
                        nc.vector.tensor_copy(
                            out=wb[:, h * HD : (h + 1) * HD], in_=ws_
                        )
                return wb

            for ns in range(NS):
                cts = (2 * ns, 2 * ns + 1)
                wbs = [load_w_tile(ct) for ct in cts]
                # d-major W tiles via DMA XBAR transpose (SBUF -> SBUF)
                wts = []
                for j, ct in enumerate(cts):
                    wt = wtp.tile([P, KC, P], bf16, tag="wt", name=f"wt{ct}")
                    nc.scalar.dma_start(wt[:, :, :], wbs[j][:, :], transpose=True)
                    wts.append(wt)
                pss = [
                    mmp.tile([P, b_sh], f32, tag="mm", name=f"ps{ct}")
                    for ct in cts
                ]
                for k in range(KC):
                    for j in range(2):
                        nc.tensor.matmul(
                            pss[j],
                            lhsT=wts[j][:, k, :],
                            rhs=xT[:, k, :],
                            start=(k == 0),
                            stop=(k == KC - 1),
                        )

                for j, ct in enumerate(cts):
                    # ||w||^2 from bf16, in place (wb dead after transposes)
                    nc.scalar.activation(
                        wbs[j], wbs[j], AF.Square,
                        accum_out=ws_negD[:, ct : ct + 1],
                    )
                    nc.vector.tensor_scalar_mul(
                        ws_negD[:, ct : ct + 1], ws_negD[:, ct : ct + 1],
                        -1.0 / d,
                    )
                    ot = outp.tile([P, b_sh], f32, tag="out")
                    nc.vector.tensor_scalar(
                        ot, pss[j], 2.0 / d, ws_negD[:, ct : ct + 1],
                        mybir.AluOpType.mult, mybir.AluOpType.add,
                    )
                    nc.vector.tensor_tensor(
                        ot, ot, xs_rep, mybir.AluOpType.add
                    )
                    nc.gpsimd.dma_start(o_d[ct * P : (ct + 1) * P, :], ot)

    nc.finalize()
    return nc


def _get_nc(b_sh, d, c):
    key = (b_sh, d, c)
    if key not in _nc_cache:
        _nc_cache[key] = _build_bass(b_sh, d, c)
    return _nc_cache[key]


last_result = None


def kernel(x, W):
    global last_result
    from concourse.bass_utils import run_bass_kernel_spmd

    x = np.ascontiguousarray(x, dtype=np.float32)
    W = np.ascontiguousarray(W, dtype=np.float32)
    b_sh = x.shape[0] // NCORES
    nc = _get_nc(b_sh, x.shape[1], W.shape[0])
    ident = np.eye(P, dtype=ml_dtypes.bfloat16)
    in_maps = [
        {
            "x": np.ascontiguousarray(x[i * b_sh : (i + 1) * b_sh]),
            "W": W,
            "ident": ident,
        }
        for i in range(NCORES)
    ]
    kw = {}
    if os.environ.get("KERNEL_TRACE", "0") == "1":
        cores = os.environ.get("KERNEL_TRACE_CORES", "0")
        kw = dict(trace=True, trace_cores=[int(t) for t in cores.split(",")])
    res = run_bass_kernel_spmd(nc, in_maps, core_ids=list(range(NCORES)), **kw)
    last_result = res
    # per-core results are out^T [C, b_sh]; assemble and transpose back
    outT = np.concatenate([res.results[i]["out"] for i in range(NCORES)], axis=1)
    return np.ascontiguousarray(outT.T)


# revision 29
# speedup vs baseline: 1.1884x; 1.1884x over previous
"""Trainium2 Bass kernel for nn_EuclideanDeconf (retrieval_knn).

Computes out = -mean((x[:, :, None] - W.T[None, :, :])**2, axis=1)
            = (2*x@W.T - ||x||^2 - ||w||^2) / D

Sharding: data-parallel over batch across 8 NeuronCores (512 rows each),
W replicated. Per core the kernel computes out^T [C, B_sh] (the host
transposes back):
  - x loaded fp32 (for exact ||x||^2), cast to bf16, PE-transposed into a
    resident d-major xT [128, 32, 512] used as the matmul moving operand
  - W streamed: cast to bf16, each 128x128 block PE-transposed and used
    immediately as the stationary operand (no resident W^T)
  - cross term accumulated in fp32 PSUM as out^T tiles [128 c, 512 b]
  - epilogue per c-tile: (2/D)*psum - ||w_c||^2/D (per-partition scalar)
    - ||x||^2/D (broadcast row, built once via a tiny DRAM roundtrip +
    gpsimd partition_broadcast)
"""

import os

import ml_dtypes
import numpy as np

B, D, C = 4096, 4096, 1024
NCORES = 8
P = 128
KG = 4         # transpose k-chunks per PSUM batch (x side)
LOOKAHEAD = 6  # W-chunk pipeline depth before its matmuls

_nc_cache = {}


def _build_bass(b_sh, d, c):
    import concourse.bacc as bacc
    import concourse.mybir as mybir
    import concourse.tile as tile

    f32 = mybir.dt.float32
    bf16 = mybir.dt.bfloat16
    AF = mybir.ActivationFunctionType

    BT = b_sh // P    # b-tiles per core
    KC = d // P       # contraction chunks
    CT = c // P       # c-tiles
    NS = CT // 2      # c-tile pairs (one pass each)
    NKG = KC // KG
    HD = d // 2

    nc = bacc.Bacc(trn_type="TRN2")
    x_d = nc.dram_tensor("x", [b_sh, d], f32, kind="ExternalInput")
    w_d = nc.dram_tensor("W", [c, d], f32, kind="ExternalInput")
    id_d = nc.dram_tensor("ident", [P, P], bf16, kind="ExternalInput")
    o_d = nc.dram_tensor("out", [c, b_sh], f32, kind="ExternalOutput")

    with tile.TileContext(nc) as tc:
        with (
            tc.tile_pool(name="const", bufs=1) as constp,
            tc.tile_pool(name="persist", bufs=1) as persist,
            tc.tile_pool(name="xstage", bufs=2) as xstage,
            tc.tile_pool(name="wstage", bufs=4) as wstage,
            tc.tile_pool(name="bfs", bufs=5) as bfs,
            tc.tile_pool(name="wtp", bufs=36) as wtp,
            tc.tile_pool(name="outp", bufs=3) as outp,
            tc.tile_pool(name="trp", bufs=4, space="PSUM") as trp,
            tc.tile_pool(name="mmp", bufs=4, space="PSUM") as mmp,
            tc.tile_pool(name="dramp", bufs=2, space="DRAM") as dramp,
        ):
            ident = constp.tile([P, P], bf16)
            with tc.high_priority():
                nc.sync.dma_start(ident[:, :], id_d[:, :])
            ws_negD = constp.tile([P, CT], f32)   # -||w_c||^2/D, c-partition
            xs_cols = constp.tile([P, BT], f32)   # ||x_b||^2, b-partition
            xsrow = constp.tile([1, b_sh], f32)   # -||x||^2/D, b-major row
            xs_rep = persist.tile([P, b_sh], f32)  # xsrow on all partitions
            xT = persist.tile([P, KC, b_sh], bf16)

            # ---- x: load, square, cast, transpose into resident xT ----
            for bt in range(BT):
                with tc.high_priority():
                    xs = xstage.tile([P, d], f32, tag="xs")
                    nc.sync.dma_start(xs, x_d[bt * P : (bt + 1) * P, :])
                    xb = bfs.tile([P, d], bf16, tag="bfs")
                    for q in range(4):
                        nc.vector.tensor_copy(
                            out=xb[:, q * (d // 4) : (q + 1) * (d // 4)],
                            in_=xs[:, q * (d // 4) : (q + 1) * (d // 4)],
                        )
                    # square in place (fp32 dead after), free-dim sum
                    nc.scalar.activation(
                        xs, xs, AF.Square, accum_out=xs_cols[:, bt : bt + 1]
                    )
                for kg in range(NKG):
                    pt = trp.tile([P, KG, P], bf16, tag="tr")
                    for j in range(KG):
                        kk = kg * KG + j
                        nc.tensor.transpose(
                            pt[:, j, :], xb[:, kk * P : (kk + 1) * P], ident
                        )
                    dst = xT[:, kg * KG : (kg + 1) * KG, bt * P : (bt + 1) * P]
                    nc.vector.tensor_copy(out=dst, in_=pt[:, :, :])

            # ---- xs row: cols -> -xs/D -> DRAM -> row -> broadcast ----
            with tc.high_priority():
                nc.vector.tensor_scalar_mul(xs_cols, xs_cols, -1.0 / d)
                dtmp = dramp.tile([BT, P], f32)
                for t in range(BT):
                    nc.gpsimd.dma_start(dtmp[t, :], xs_cols[:, t : t + 1])
                nc.gpsimd.dma_start(xsrow[0:1, :], dtmp[:, :])
                nc.gpsimd.partition_broadcast(xs_rep[:, :], xsrow[0:1, :])

            # ---- W: stream pair-of-c-tiles passes ----
            def load_w_tile(ct):
                wb = bfs.tile([P, d], bf16, tag="bfs", name=f"wb{ct}")
                for h in range(2):
                    with tc.high_priority():
                        ws_ = wstage.tile([P, HD], f32, tag="ws")
                        nc.sync.dma_start(
                            ws_, w_d[ct * P : (ct + 1) * P, h * HD : (h + 1) * HD]
                        )
                        for q in range(2):
                            QW = HD // 2
                            nc.vector.tensor_copy(
                                out=wb[:, h * HD + q * QW : h * HD + (q + 1) * QW],
                                in_=ws_[:, q * QW : (q + 1) * QW],
                            )
                return wb

            all_wbs = {}

            def ensure_loaded(ns):
                if ns < NS and ns not in all_wbs:
                    all_wbs[ns] = [load_w_tile(ct) for ct in (2 * ns, 2 * ns + 1)]

            all_chunks = {}

            def emit_chunk(ns, k):
                wbs = all_wbs[ns]
                pt = trp.tile([P, 2, P], bf16, tag="tr")
                for j in range(2):
                    nc.tensor.transpose(
                        pt[:, j, :], wbs[j][:, k * P : (k + 1) * P], ident
                    )
                wt = wtp.tile([P, 2, P], bf16, tag="wt")
                if k % 2 == 1:
                    nc.scalar.copy(wt[:, :, :], pt[:, :, :])
                else:
                    nc.vector.tensor_copy(out=wt[:, :, :], in_=pt[:, :, :])
                all_chunks.setdefault(ns, []).append(wt)

            ensure_loaded(0)
            for k in range(KC):
                emit_chunk(0, k)

            for ns in range(NS):
                cts = (2 * ns, 2 * ns + 1)
                ensure_loaded(ns + 1)
                wbs = all_wbs[ns]
                pss = [
                    mmp.tile([P, b_sh], f32, tag="mm", name=f"ps{ct}")
                    for ct in cts
                ]
                for k in range(KC):
                    if ns + 1 < NS:
                        emit_chunk(ns + 1, k)
                    for j in range(2):
                        nc.tensor.matmul(
                            pss[j],
                            lhsT=all_chunks[ns][k][:, j, :],
                            rhs=xT[:, k, :],
                            start=(k == 0),
                            stop=(k == KC - 1),
                        )

                for j, ct in enumerate(cts):
                    # ||w||^2 from bf16, in place (wb dead after transposes)
                    nc.scalar.activation(
                        wbs[j], wbs[j], AF.Square,
                        accum_out=ws_negD[:, ct : ct + 1],
                    )
                    nc.vector.tensor_scalar_mul(
                        ws_negD[:, ct : ct + 1], ws_negD[:, ct : ct + 1],
                        -1.0 / d,
                    )
                    ot = outp.tile([P, b_sh], f32, tag="out")
                    nc.vector.tensor_scalar(
                        ot, pss[j], 2.0 / d, ws_negD[:, ct : ct + 1],
                        mybir.AluOpType.mult, mybir.AluOpType.add,
                    )
                    nc.vector.tensor_tensor(
                        ot, ot, xs_rep, mybir.AluOpType.add
                    )
                    nc.gpsimd.dma_start(o_d[ct * P : (ct + 1) * P, :], ot)

    nc.finalize()
    return nc


def _get_nc(b_sh, d, c):
    key = (b_sh, d, c)
    if key not in _nc_cache:
        _nc_cache[key] = _build_bass(b_sh, d, c)
    return _nc_cache[key]


last_result = None


def kernel(x, W):
    global last_result
    from concourse.bass_utils import run_bass_kernel_spmd

    x = np.ascontiguousarray(x, dtype=np.float32)
    W = np.ascontiguousarray(W, dtype=np.float32)
    b_sh = x.shape[0] // NCORES
    nc = _get_nc(b_sh, x.shape[1], W.shape[0])
    ident = np.eye(P, dtype=ml_dtypes.bfloat16)
    in_maps = [
        {
            "x": np.ascontiguousarray(x[i * b_sh : (i + 1) * b_sh]),
            "W": W,
            "ident": ident,
        }
        for i in range(NCORES)
    ]
    kw = {}
    if os.environ.get("KERNEL_TRACE", "0") == "1":
        cores = os.environ.get("KERNEL_TRACE_CORES", "0")
        kw = dict(trace=True, trace_cores=[int(t) for t in cores.split(",")])
    res = run_bass_kernel_spmd(nc, in_maps, core_ids=list(range(NCORES)), **kw)
    last_result = res
    # per-core results are out^T [C, b_sh]; assemble and transpose back
    outT = np.concatenate([res.results[i]["out"] for i in range(NCORES)], axis=1)
    return np.ascontiguousarray(outT.T)


# revision 30
# speedup vs baseline: 1.3063x; 1.0992x over previous
"""Trainium2 Bass kernel for nn_EuclideanDeconf (retrieval_knn).

Computes out = -mean((x[:, :, None] - W.T[None, :, :])**2, axis=1)
            = (2*x@W.T - ||x||^2 - ||w||^2) / D

Sharding: data-parallel over batch across 8 NeuronCores (512 rows each),
W replicated. Per core the kernel computes out^T [C, B_sh] (the host
transposes back):
  - x loaded fp32 (for exact ||x||^2), cast to bf16, PE-transposed into a
    resident d-major xT [128, 32, 512] used as the matmul moving operand
  - W streamed: cast to bf16, each 128x128 block PE-transposed and used
    immediately as the stationary operand (no resident W^T)
  - cross term accumulated in fp32 PSUM as out^T tiles [128 c, 512 b]
  - epilogue per c-tile: (2/D)*psum - ||w_c||^2/D (per-partition scalar)
    - ||x||^2/D (broadcast row, built once via a tiny DRAM roundtrip +
    gpsimd partition_broadcast)
"""

import os

import ml_dtypes
import numpy as np

B, D, C = 4096, 4096, 1024
NCORES = 8
P = 128
KG = 4         # transpose k-chunks per PSUM batch (x side)
LOOKAHEAD = 6  # W-chunk pipeline depth before its matmuls

_nc_cache = {}


def _build_bass(b_sh, d, c):
    import concourse.bacc as bacc
    import concourse.mybir as mybir
    import concourse.tile as tile

    f32 = mybir.dt.float32
    bf16 = mybir.dt.bfloat16
    AF = mybir.ActivationFunctionType

    BT = b_sh // P    # b-tiles per core
    KC = d // P       # contraction chunks
    CT = c // P       # c-tiles
    NS = CT // 2      # c-tile pairs (one pass each)
    NKG = KC // KG
    HD = d // 2

    nc = bacc.Bacc(trn_type="TRN2")
    x_d = nc.dram_tensor("x", [b_sh, d], f32, kind="ExternalInput")
    w_d = nc.dram_tensor("W", [c, d], f32, kind="ExternalInput")
    id_d = nc.dram_tensor("ident", [P, P], bf16, kind="ExternalInput")
    o_d = nc.dram_tensor("out", [c, b_sh], f32, kind="ExternalOutput")

    with tile.TileContext(nc) as tc:
        with (
            tc.tile_pool(name="const", bufs=1) as constp,
            tc.tile_pool(name="persist", bufs=1) as persist,
            tc.tile_pool(name="xstage", bufs=2) as xstage,
            tc.tile_pool(name="wstage", bufs=6) as wstage,
            tc.tile_pool(name="bfs", bufs=5) as bfs,
            tc.tile_pool(name="wtp", bufs=36) as wtp,
            tc.tile_pool(name="outp", bufs=3) as outp,
            tc.tile_pool(name="trp", bufs=4, space="PSUM") as trp,
            tc.tile_pool(name="mmp", bufs=4, space="PSUM") as mmp,
            tc.tile_pool(name="dramp", bufs=2, space="DRAM") as dramp,
        ):
            ident = constp.tile([P, P], bf16)
            with tc.high_priority():
                nc.sync.dma_start(ident[:, :], id_d[:, :])
            ws_negD = constp.tile([P, CT], f32)   # -||w_c||^2/D, c-partition
            xs_cols = constp.tile([P, BT], f32)   # ||x_b||^2, b-partition
            xsrow = constp.tile([1, b_sh], f32)   # -||x||^2/D, b-major row
            xs_rep = persist.tile([P, b_sh], f32)  # xsrow on all partitions
            xT = persist.tile([P, KC, b_sh], bf16)

            # ---- x: load, square, cast, transpose into resident xT ----
            for bt in range(BT):
                with tc.high_priority():
                    xs = xstage.tile([P, d], f32, tag="xs")
                    nc.sync.dma_start(xs, x_d[bt * P : (bt + 1) * P, :])
                    xb = bfs.tile([P, d], bf16, tag="bfs")
                    for q in range(4):
                        nc.vector.tensor_copy(
                            out=xb[:, q * (d // 4) : (q + 1) * (d // 4)],
                            in_=xs[:, q * (d // 4) : (q + 1) * (d // 4)],
                        )
                    # square in place (fp32 dead after), free-dim sum
                    nc.scalar.activation(
                        xs, xs, AF.Square, accum_out=xs_cols[:, bt : bt + 1]
                    )
                for kg in range(NKG):
                    pt = trp.tile([P, KG, P], bf16, tag="tr")
                    for j in range(KG):
                        kk = kg * KG + j
                        nc.tensor.transpose(
                            pt[:, j, :], xb[:, kk * P : (kk + 1) * P], ident
                        )
                    dst = xT[:, kg * KG : (kg + 1) * KG, bt * P : (bt + 1) * P]
                    nc.vector.tensor_copy(out=dst, in_=pt[:, :, :])

            # ---- xs row: cols -> -xs/D -> DRAM -> row -> broadcast ----
            with tc.high_priority():
                nc.vector.tensor_scalar_mul(xs_cols, xs_cols, -1.0 / d)
                dtmp = dramp.tile([BT, P], f32)
                for t in range(BT):
                    nc.gpsimd.dma_start(dtmp[t, :], xs_cols[:, t : t + 1])
                nc.gpsimd.dma_start(xsrow[0:1, :], dtmp[:, :])
                nc.gpsimd.partition_broadcast(xs_rep[:, :], xsrow[0:1, :])

            # ---- W: stream pair-of-c-tiles passes ----
            def load_w_tile(ct):
                wb = bfs.tile([P, d], bf16, tag="bfs", name=f"wb{ct}")
                for h in range(2):
                    with tc.high_priority():
                        ws_ = wstage.tile([P, HD], f32, tag="ws")
                        nc.sync.dma_start(
                            ws_, w_d[ct * P : (ct + 1) * P, h * HD : (h + 1) * HD]
                        )
                        for q in range(2):
                            QW = HD // 2
                            nc.vector.tensor_copy(
                                out=wb[:, h * HD + q * QW : h * HD + (q + 1) * QW],
                                in_=ws_[:, q * QW : (q + 1) * QW],
                            )
                return wb

            all_wbs = {}

            def ensure_loaded(ns):
                if ns < NS and ns not in all_wbs:
                    all_wbs[ns] = [load_w_tile(ct) for ct in (2 * ns, 2 * ns + 1)]

            all_chunks = {}

            def emit_chunk(ns, k):
                wbs = all_wbs[ns]
                pt = trp.tile([P, 2, P], bf16, tag="tr")
                for j in range(2):
                    nc.tensor.transpose(
                        pt[:, j, :], wbs[j][:, k * P : (k + 1) * P], ident
                    )
                wt = wtp.tile([P, 2, P], bf16, tag="wt")
                nc.scalar.copy(wt[:, :, :], pt[:, :, :])
                all_chunks.setdefault(ns, []).append(wt)

            ensure_loaded(0)
            for k in range(KC):
                emit_chunk(0, k)

            for ns in range(NS):
                cts = (2 * ns, 2 * ns + 1)
                ensure_loaded(ns + 1)
                wbs = all_wbs[ns]
                pss = [
                    mmp.tile([P, b_sh], f32, tag="mm", name=f"ps{ct}")
                    for ct in cts
                ]
                for k in range(KC):
                    if ns + 1 < NS:
                        emit_chunk(ns + 1, k)
                    for j in range(2):
                        nc.tensor.matmul(
                            pss[j],
                            lhsT=all_chunks[ns][k][:, j, :],
                            rhs=xT[:, k, :],
                            start=(k == 0),
                            stop=(k == KC - 1),
                        )

                for j, ct in enumerate(cts):
                    # ||w||^2 from bf16, in place (wb dead after transposes)
                    nc.scalar.activation(
                        wbs[j], wbs[j], AF.Square,
                        accum_out=ws_negD[:, ct : ct + 1],
                    )
                    nc.vector.tensor_scalar_mul(
                        ws_negD[:, ct : ct + 1], ws_negD[:, ct : ct + 1],
                        -1.0 / d,
                    )
                    ot = outp.tile([P, b_sh], f32, tag="out")
                    nc.vector.tensor_scalar(
                        ot, pss[j], 2.0 / d, ws_negD[:, ct : ct + 1],
                        mybir.AluOpType.mult, mybir.AluOpType.add,
                    )
                    nc.vector.tensor_tensor(
                        ot, ot, xs_rep, mybir.AluOpType.add
                    )
                    nc.gpsimd.dma_start(o_d[ct * P : (ct + 1) * P, :], ot)

    nc.finalize()
    return nc


def _get_nc(b_sh, d, c):
    key = (b_sh, d, c)
    if key not in _nc_cache:
        _nc_cache[key] = _build_bass(b_sh, d, c)
    return _nc_cache[key]


last_result = None


def kernel(x, W):
    global last_result
    from concourse.bass_utils import run_bass_kernel_spmd

    x = np.ascontiguousarray(x, dtype=np.float32)
    W = np.ascontiguousarray(W, dtype=np.float32)
    b_sh = x.shape[0] // NCORES
    nc = _get_nc(b_sh, x.shape[1], W.shape[0])
    ident = np.eye(P, dtype=ml_dtypes.bfloat16)
    in_maps = [
        {
            "x": np.ascontiguousarray(x[i * b_sh : (i + 1) * b_sh]),
            "W": W,
            "ident": ident,
        }
        for i in range(NCORES)
    ]
    kw = {}
    if os.environ.get("KERNEL_TRACE", "0") == "1":
        cores = os.environ.get("KERNEL_TRACE_CORES", "0")
        kw = dict(trace=True, trace_cores=[int(t) for t in cores.split(",")])
    res = run_bass_kernel_spmd(nc, in_maps, core_ids=list(range(NCORES)), **kw)
    last_result = res
    # per-core results are out^T [C, b_sh]; assemble and transpose back
    outT = np.concatenate([res.results[i]["out"] for i in range(NCORES)], axis=1)
    return np.ascontiguousarray(outT.T)
